# revision 3
# baseline (speedup 1.0000x reference)
"""CRF loss (forward-algorithm log-partition + gold-path score) on 8 Trainium2
NeuronCores — bidirectional-recursion version.

Algorithm (per batch row):
  log_den = logsumexp over tag paths; loss = mean_b(log_den - log_num).

Key structure vs the single-direction version:
  * The forward recursion over S=512 steps is split into a forward half
    (alpha from t=0) and an independent backward half (beta from t=511),
    meeting in the middle:  Z = q_{255}^T Ê r_{256}.  That halves the
    serial chain to 255 matmul->multiply rounds.
  * Both directions share ONE stationary matrix diag(Ê, Ê, Êᵀ, Êᵀ)
    (32-padded 24x24 blocks), so the PE never reloads weights:
      partitions  0- 63: forward tags for row-sets 0-127 / 128-255
      partitions 64-127: backward tags for the same row-sets
    One [128x128]@[128,128] matmul + one [128,128] DVE multiply advances
    all 256 rows of the core one step in BOTH directions.
  * Rescale: F_t = exp(e_t - c0), c0 = 6 ln2; ln Z = ln(sum z) + S*c0.
  * Gold score emission part: sum(ep * one_hot) via GpSimd multiply +
    full-tensor reduce, overlapped with the recursion.
  * Gold score transition part only needs `tags` and the 24x24 table, so
    it is computed on host from a bigram histogram (index preprocessing,
    like the one-hot encoding).

Host side reshapes/pads/one-hot-encodes inputs into the packed
[block*32+tag, step*128+col] layout (backward blocks time-reversed).
"""

import math

import numpy as np
import ml_dtypes

import concourse.bass as bass
import concourse.bacc as bacc
import concourse.tile as tile
import concourse.mybir as mybir
import concourse.bass_utils as bass_utils
from concourse.bass_utils import run_bass_kernel_spmd

BF16 = mybir.dt.bfloat16
F32 = mybir.dt.float32
AF = mybir.ActivationFunctionType
ALU = mybir.AluOpType
NPBF16 = ml_dtypes.bfloat16

B, S, NT = 2048, 512, 24
JP = 32                    # padded tag dim (multiple of 32)
NCORES = 8
RPC = B // NCORES          # rows per core (256)
M = S // 2                 # steps per direction (256)
FD = 128                   # free dim: 128 row-slots per partition-block
CS = 32                    # steps per streamed chunk
NCH = M // CS              # chunks (8)
C0 = 6.0 * math.log(2.0)   # per-step rescale of the partition chain
NEG = -30000.0             # pad value; exp -> 0


def build_program():
    nc = bacc.Bacc(trn_type="TRN2")
    ep_d = nc.dram_tensor("ep", [128, M * FD], BF16, kind="ExternalInput")
    hp_d = nc.dram_tensor("hp", [128, M * FD], BF16, kind="ExternalInput")
    etr_d = nc.dram_tensor("etr", [NT, NT], BF16, kind="ExternalInput")
    etrt_d = nc.dram_tensor("etrt", [NT, NT], BF16, kind="ExternalInput")
    out_d = nc.dram_tensor("out", [1, 1], F32, kind="ExternalOutput")

    with tile.TileContext(nc) as tc:
        with tc.tile_pool(name="const", bufs=1) as const, \
             tc.tile_pool(name="chunks", bufs=3) as chunks, \
             tc.tile_pool(name="prods", bufs=2) as prods, \
             tc.tile_pool(name="state", bufs=1) as state, \
             tc.tile_pool(name="small", bufs=2) as small, \
             tc.tile_pool(name="pmm", bufs=2, space="PSUM") as pmm, \
             tc.tile_pool(name="psg", bufs=1, space="PSUM") as psg:

            # ---- constants (memset-only parts; DMA parts deferred) ----
            wbd = const.tile([128, 128], BF16)
            nc.vector.memset(wbd[:, :], 0.0)
            sel64 = const.tile([64, 2], F32)
            nc.vector.memset(sel64[:, :], 0.0)
            nc.vector.memset(sel64[0:32, 0:1], 1.0)
            nc.vector.memset(sel64[32:64, 1:2], 1.0)
            ones2 = const.tile([2, 1], F32)
            nc.vector.memset(ones2[:, :], 1.0)
            negc0 = const.tile([128, 1], F32)
            nc.vector.memset(negc0[:, :], -C0)

            # ---- persistent state ----
            q = state.tile([128, FD], BF16, name="q")
            gparts = state.tile([1, NCH], F32, name="gparts")
            nc.vector.memset(gparts[:, :], 0.0)

            ep_tiles = [None] * NCH
            hp_tiles = [None] * NCH
            fh_tiles = [None] * NCH

            def prep_chunk(k):
                ep_t = chunks.tile([128, CS, FD], BF16, tag="ep",
                                   name=f"ep_{k}")
                hp_t = chunks.tile([128, CS, FD], BF16, tag="hp",
                                   name=f"hp_{k}")
                lo = k * CS * FD
                nc.sync.dma_start(
                    out=ep_t[:, :, :],
                    in_=ep_d[:, lo:lo + CS * FD]
                    .rearrange("p (s c) -> p s c", c=FD))
                nc.sync.dma_start(
                    out=hp_t[:, :, :],
                    in_=hp_d[:, lo:lo + CS * FD]
                    .rearrange("p (s c) -> p s c", c=FD))
                fh = chunks.tile([128, CS, FD], BF16, tag="fh",
                                 name=f"fh_{k}")
                # F_hat = exp(ep - c0); split so early steps unblock sooner
                for e4 in range(4):
                    ssl = slice(e4 * CS // 4, (e4 + 1) * CS // 4)
                    nc.scalar.activation(fh[:, ssl, :], ep_t[:, ssl, :],
                                         AF.Exp, bias=negc0[:, :])
                ep_tiles[k] = ep_t
                hp_tiles[k] = hp_t
                fh_tiles[k] = fh

            def gold_chunk(k):
                # emission gold partial: sum(ep * hp) over the whole chunk
                prod = prods.tile([128, CS * FD], BF16, tag="prod",
                                  name=f"prod_{k}")
                nc.gpsimd.tensor_mul(
                    prod[:, :],
                    ep_tiles[k][:, :, :].rearrange("p s c -> p (s c)"),
                    hp_tiles[k][:, :, :].rearrange("p s c -> p (s c)"))
                nc.gpsimd.reduce_sum(gparts[0:1, k:k + 1], prod[:, :],
                                     axis=mybir.AxisListType.XYZWC)

            # ---- pipeline ----
            prep_chunk(0)
            prep_chunk(1)
            # deferred constant DMAs (queue behind the chunk-0 loads that
            # gate the first recursion rounds)
            e24 = const.tile([NT, NT], BF16)
            nc.sync.dma_start(out=e24[:, :], in_=etr_d[:, :])
            et24 = const.tile([NT, NT], BF16)
            nc.sync.dma_start(out=et24[:, :], in_=etrt_d[:, :])
            for g in range(4):
                src = e24 if g < 2 else et24
                nc.sync.dma_start(
                    out=wbd[32 * g:32 * g + NT, 32 * g:32 * g + NT],
                    in_=src[:, :])
            # q init: step-0 factors for both directions
            nc.vector.tensor_copy(q[:, :], fh_tiles[0][:, 0, :])

            for k in range(NCH):
                if k + 2 < NCH:
                    prep_chunk(k + 2)
                gold_chunk(k)
                s_lo = 1 if k == 0 else 0
                for sl in range(s_lo, CS):
                    p_t = pmm.tile([128, FD], F32, tag="p",
                                   name=f"p_{k}_{sl}")
                    nc.tensor.matmul(p_t[:, :], wbd[:, :], q[:, :],
                                     start=True, stop=True)
                    nc.vector.tensor_mul(q[:, :], p_t[:, :],
                                         fh_tiles[k][:, sl, :])

            # ---- finalization:  Z = q_fwd^T Ê r_bwd  per row ----
            pz = pmm.tile([128, FD], F32, tag="p", name="pz")
            nc.tensor.matmul(pz[:, :], wbd[:, :], q[:, :],
                             start=True, stop=True)
            qsh = small.tile([64, FD], BF16, tag="qsh", name="qsh")
            nc.sync.dma_start(out=qsh[:, :], in_=q[64:128, :])
            z = small.tile([64, FD], F32, tag="z", name="z")
            nc.vector.tensor_mul(z[:, :], pz[0:64, :], qsh[:, :])
            zz = psg.tile([2, FD], F32, tag="fin", name="zz")
            nc.tensor.matmul(zz[:, :], sel64[:, :], z[:, :],
                             start=True, stop=True)
            lnz = small.tile([2, FD], F32, tag="lnz", name="lnz")
            nc.scalar.activation(lnz[:, :], zz[:, :], AF.Ln)
            lnr = small.tile([2, 1], F32, tag="lnr", name="lnr")
            nc.vector.reduce_sum(lnr[:, :], lnz[:, :],
                                 axis=mybir.AxisListType.X)
            zt = psg.tile([1, 1], F32, tag="fin2", name="zt")
            nc.tensor.matmul(zt[:, :], ones2[:, :], lnr[:, :],
                             start=True, stop=True)

            gsum = small.tile([1, 1], F32, tag="gsum", name="gsum")
            nc.vector.reduce_sum(gsum[:, :], gparts[:, :],
                                 axis=mybir.AxisListType.X)
            gneg = small.tile([1, 1], F32, tag="gneg", name="gneg")
            nc.scalar.mul(gneg[:, :], gsum[:, :], -1.0)

            sl_t = small.tile([1, 1], F32, tag="outv", name="sl_t")
            nc.scalar.copy(sl_t[:, :], zt[:, :])
            outv = small.tile([1, 1], F32, tag="outv", name="outv")
            # + per-core constant: RPC rows * S steps * c0
            nc.vector.scalar_tensor_tensor(
                outv[:, :], sl_t[:, :], float(RPC) * float(S) * C0,
                gneg[:, :], ALU.add, ALU.add)
            nc.sync.dma_start(out=out_d[:, :], in_=outv[:, :])
    nc.compile()
    return nc


def prep_inputs(emissions, tags, transition_scores):
    """Host-side layout prep -> per-core input maps.

    Packed layout (per core, 256 rows):
      part = blk*32 + j, col = s*128 + b, where
      blk 0: fwd rows   0-127   value x[row=b,     t=s,     tag j]
      blk 1: fwd rows 128-255   value x[row=128+b, t=s,     tag j]
      blk 2: bwd rows   0-127   value x[row=b,     t=511-s, tag j]
      blk 3: bwd rows 128-255   value x[row=128+b, t=511-s, tag j]
    ep holds emissions (pad NEG), hp the one-hot of tags (pad 0).
    """
    e = np.asarray(emissions)
    t = np.asarray(tags)
    ep = np.full((B, S, JP), NEG, dtype=NPBF16)
    ep[:, :, :NT] = e.astype(NPBF16)
    hp = np.zeros((B, S, JP), dtype=NPBF16)
    np.put_along_axis(hp, t[..., None], np.asarray(1.0, NPBF16), axis=2)

    def pack(x):
        x = x.reshape(NCORES, 2, 128, S, JP)      # [core, half, b, s, j]
        fwd = x[:, :, :, :M, :]                   # [core, half, b, s, j]
        bwd = x[:, :, :, ::-1, :][:, :, :, :M, :]
        X = np.stack([fwd[:, 0], fwd[:, 1], bwd[:, 0], bwd[:, 1]],
                     axis=1)                      # [core, blk, b, s, j]
        X = np.ascontiguousarray(X.transpose(0, 1, 4, 3, 2))
        return X.reshape(NCORES, 128, M * FD)     # part=blk*32+j, col=s*128+b

    epk, hpk = pack(ep), pack(hp)
    tr64 = np.asarray(transition_scores, dtype=np.float64)
    etr = np.exp(tr64).astype(NPBF16)
    etrt = np.ascontiguousarray(etr.T)
    return [
        {"ep": np.ascontiguousarray(epk[c]), "hp": np.ascontiguousarray(hpk[c]),
         "etr": etr, "etrt": etrt}
        for c in range(NCORES)
    ]


def host_trans_gold(tags, transition_scores):
    """Gold transition score summed over all rows: tags-only bigram
    histogram dotted with the 24x24 table (exact, fp64)."""
    t = np.asarray(tags).astype(np.int64)
    pairs = t[:, :-1] * NT + t[:, 1:]
    counts = np.bincount(pairs.ravel(), minlength=NT * NT).astype(np.float64)
    tr64 = np.asarray(transition_scores, dtype=np.float64)
    return float((counts * tr64.ravel()).sum())


def combine(partials, trans_gold):
    return np.float32((sum(partials) - trans_gold) / B)


_PROGRAM_CACHE = {}


def kernel(emissions, tags, mask, transition_scores):
    assert np.asarray(mask).min() == 1, "kernel assumes all-ones mask"
    in_maps = prep_inputs(emissions, tags, transition_scores)
    tg = host_trans_gold(tags, transition_scores)

    if "nc" not in _PROGRAM_CACHE:
        _PROGRAM_CACHE["nc"] = build_program()
    nc = _PROGRAM_CACHE["nc"]

    res = run_bass_kernel_spmd(nc, in_maps, core_ids=list(range(NCORES)))
    partials = [float(r["out"][0, 0]) for r in res.results]
    return combine(partials, tg)


# revision 11
# speedup vs baseline: 1.4709x; 1.4709x over previous
"""CRF loss (forward-algorithm log-partition + gold-path score) on 8 Trainium2
NeuronCores — bidirectional-recursion version.

Algorithm (per batch row):
  log_den = logsumexp over tag paths; loss = mean_b(log_den - log_num).

Key structure vs the single-direction version:
  * The forward recursion over S=512 steps is split into a forward half
    (alpha from t=0) and an independent backward half (beta from t=511),
    meeting in the middle:  Z = q_{255}^T Ê r_{256}.  That halves the
    serial chain to 255 matmul->multiply rounds.
  * Both directions share ONE stationary matrix diag(Ê, Ê, Êᵀ, Êᵀ)
    (32-padded 24x24 blocks), so the PE never reloads weights:
      partitions  0- 63: forward tags for row-sets 0-127 / 128-255
      partitions 64-127: backward tags for the same row-sets
    One [128x128]@[128,128] matmul + one [128,128] DVE multiply advances
    all 256 rows of the core one step in BOTH directions.
  * Rescale: F_t = exp(e_t - c0), c0 = 6 ln2; ln Z = ln(sum z) + S*c0.
  * Gold score emission part: sum(ep * one_hot) via GpSimd multiply +
    full-tensor reduce, overlapped with the recursion.
  * Gold score transition part only needs `tags` and the 24x24 table, so
    it is computed on host from a bigram histogram (index preprocessing,
    like the one-hot encoding).

Host side reshapes/pads/one-hot-encodes inputs into the packed
[block*32+tag, step*128+col] layout (backward blocks time-reversed).
"""

import math

import numpy as np
import ml_dtypes

import concourse.bass as bass
import concourse.bacc as bacc
import concourse.tile as tile
import concourse.mybir as mybir
import concourse.bass_utils as bass_utils
from concourse.bass_utils import run_bass_kernel_spmd

BF16 = mybir.dt.bfloat16
F32 = mybir.dt.float32
AF = mybir.ActivationFunctionType
ALU = mybir.AluOpType
NPBF16 = ml_dtypes.bfloat16

B, S, NT = 2048, 512, 24
JP = 32                    # padded tag dim (multiple of 32)
NCORES = 8
RPC = B // NCORES          # rows per core (256)
M = S // 2                 # steps per direction (256)
FD = 128                   # free dim: 128 row-slots per partition-block
CS = 32                    # steps per streamed chunk
NCH = M // CS              # chunks (8)
C0 = 6.0 * math.log(2.0)   # per-step rescale of the partition chain
NEG = -30000.0             # pad value; exp -> 0


def build_program():
    nc = bacc.Bacc(trn_type="TRN2")
    ep_d = nc.dram_tensor("ep", [128, M * FD], BF16, kind="ExternalInput")
    hp_d = nc.dram_tensor("hp", [128, M * FD], BF16, kind="ExternalInput")
    etr_d = nc.dram_tensor("etr", [NT, NT], BF16, kind="ExternalInput")
    etrt_d = nc.dram_tensor("etrt", [NT, NT], BF16, kind="ExternalInput")
    out_d = nc.dram_tensor("out", [1, 1], F32, kind="ExternalOutput")

    with tile.TileContext(nc) as tc:
        with tc.tile_pool(name="const", bufs=1) as const, \
             tc.tile_pool(name="chunks", bufs=3) as chunks, \
             tc.tile_pool(name="prods", bufs=2) as prods, \
             tc.tile_pool(name="state", bufs=1) as state, \
             tc.tile_pool(name="small", bufs=4) as small, \
             tc.tile_pool(name="pmm", bufs=2, space="PSUM") as pmm, \
             tc.tile_pool(name="psg", bufs=1, space="PSUM") as psg:

            # ---- constants (memset-only parts; DMA parts deferred) ----
            wbd = const.tile([128, 128], BF16)
            nc.vector.memset(wbd[:, :], 0.0)
            sel64 = const.tile([64, 2], F32)
            nc.vector.memset(sel64[:, :], 0.0)
            nc.vector.memset(sel64[0:32, 0:1], 1.0)
            nc.vector.memset(sel64[32:64, 1:2], 1.0)
            ones2 = const.tile([2, 1], F32)
            nc.vector.memset(ones2[:, :], 1.0)
            onesn128 = const.tile([128, 1], F32)
            nc.vector.memset(onesn128[:, :], -1.0)
            negc0 = const.tile([128, 1], F32)
            nc.vector.memset(negc0[:, :], -C0)

            # ---- persistent state ----
            q = state.tile([128, FD], BF16, name="q")
            gparts = state.tile([128, NCH], F32, name="gparts")
            nc.vector.memset(gparts[:, :], 0.0)

            ep_tiles = [None] * NCH
            hp_tiles = [None] * NCH
            fh_tiles = [None] * NCH

            def prep_chunk(k):
                ep_t = chunks.tile([128, CS, FD], BF16, tag="ep",
                                   name=f"ep_{k}")
                hp_t = chunks.tile([128, CS, FD], BF16, tag="hp",
                                   name=f"hp_{k}")
                lo = k * CS * FD
                nc.sync.dma_start(
                    out=ep_t[:, :, :],
                    in_=ep_d[:, lo:lo + CS * FD]
                    .rearrange("p (s c) -> p s c", c=FD))
                nc.sync.dma_start(
                    out=hp_t[:, :, :],
                    in_=hp_d[:, lo:lo + CS * FD]
                    .rearrange("p (s c) -> p s c", c=FD))
                fh = chunks.tile([128, CS, FD], BF16, tag="fh",
                                 name=f"fh_{k}")
                # F_hat = exp(ep - c0); split so early steps unblock sooner
                for e4 in range(4):
                    ssl = slice(e4 * CS // 4, (e4 + 1) * CS // 4)
                    nc.scalar.activation(fh[:, ssl, :], ep_t[:, ssl, :],
                                         AF.Exp, bias=negc0[:, :])
                ep_tiles[k] = ep_t
                hp_tiles[k] = hp_t
                fh_tiles[k] = fh

            def gold_chunk(k):
                # emission gold partial: sum(ep * hp) over the whole chunk.
                # Multiply on GpSimd; per-partition reduce on ScalarE via
                # the activation accumulator (keeps the DVE recursion clean).
                prod = prods.tile([128, CS * FD], BF16, tag="prod",
                                  name=f"prod_{k}")
                nc.gpsimd.tensor_mul(
                    prod[:, :],
                    ep_tiles[k][:, :, :].rearrange("p s c -> p (s c)"),
                    hp_tiles[k][:, :, :].rearrange("p s c -> p (s c)"))
                scr = prods.tile([128, CS * FD], BF16, tag="scr",
                                 name=f"scr_{k}")
                nc.scalar.activation(scr[:, :], prod[:, :], AF.Copy,
                                     accum_out=gparts[:, k:k + 1])

            # ---- pipeline ----
            prep_chunk(0)
            prep_chunk(1)
            # deferred constant DMAs (queue behind the chunk-0 loads that
            # gate the first recursion rounds)
            e24 = const.tile([NT, NT], BF16)
            nc.sync.dma_start(out=e24[:, :], in_=etr_d[:, :])
            et24 = const.tile([NT, NT], BF16)
            nc.sync.dma_start(out=et24[:, :], in_=etrt_d[:, :])
            for g in range(4):
                src = e24 if g < 2 else et24
                nc.sync.dma_start(
                    out=wbd[32 * g:32 * g + NT, 32 * g:32 * g + NT],
                    in_=src[:, :])
            # q init: step-0 factors for both directions
            nc.vector.tensor_copy(q[:, :], fh_tiles[0][:, 0, :])

            for k in range(NCH):
                if k + 2 < NCH:
                    prep_chunk(k + 2)
                gold_chunk(k)
                s_lo = 1 if k == 0 else 0
                for sl in range(s_lo, CS):
                    p_t = pmm.tile([128, FD], F32, tag="p",
                                   name=f"p_{k}_{sl}")
                    nc.tensor.matmul(p_t[:, :], wbd[:, :], q[:, :],
                                     start=True, stop=True)
                    nc.vector.tensor_mul(q[:, :], p_t[:, :],
                                         fh_tiles[k][:, sl, :])

            # ---- finalization:  Z = q_fwd^T Ê r_bwd  per row ----
            pz = pmm.tile([128, FD], F32, tag="p", name="pz")
            nc.tensor.matmul(pz[:, :], wbd[:, :], q[:, :],
                             start=True, stop=True)
            qsh = small.tile([64, FD], BF16, tag="qsh", name="qsh")
            nc.sync.dma_start(out=qsh[:, :], in_=q[64:128, :])
            z = small.tile([64, FD], F32, tag="z", name="z")
            nc.vector.tensor_mul(z[:, :], pz[0:64, :], qsh[:, :])
            zz = psg.tile([2, FD], F32, tag="fin", name="zz")
            nc.tensor.matmul(zz[:, :], sel64[:, :], z[:, :],
                             start=True, stop=True)
            lnz = small.tile([2, FD], F32, tag="lnz", name="lnz")
            nc.scalar.activation(lnz[:, :], zz[:, :], AF.Ln)
            lnr = small.tile([2, 1], F32, tag="lnr", name="lnr")
            nc.vector.reduce_sum(lnr[:, :], lnz[:, :],
                                 axis=mybir.AxisListType.X)
            zt = psg.tile([1, 1], F32, tag="finzt", name="zt")
            nc.tensor.matmul(zt[:, :], ones2[:, :], lnr[:, :],
                             start=True, stop=True)

            g128 = small.tile([128, 1], F32, tag="g128", name="g128")
            nc.vector.reduce_sum(g128[:, :], gparts[:, :],
                                 axis=mybir.AxisListType.X)
            fing = psg.tile([1, 1], F32, tag="fin2", name="fing")
            nc.tensor.matmul(fing[:, :], onesn128[:, :], g128[:, :],
                             start=True, stop=True)

            sl_t = small.tile([1, 1], F32, tag="outv", name="sl_t")
            nc.scalar.copy(sl_t[:, :], zt[:, :])
            sg_t = small.tile([1, 1], F32, tag="outv", name="sg_t")
            nc.scalar.copy(sg_t[:, :], fing[:, :])
            outv = small.tile([1, 1], F32, tag="outv", name="outv")
            # + per-core constant: RPC rows * S steps * c0 (sg holds -gold)
            nc.vector.scalar_tensor_tensor(
                outv[:, :], sl_t[:, :], float(RPC) * float(S) * C0,
                sg_t[:, :], ALU.add, ALU.add)
            nc.sync.dma_start(out=out_d[:, :], in_=outv[:, :])
    _bacc_compile_no_ldw_split(nc)
    return nc


def _bacc_compile_no_ldw_split(nc):
    """Bacc.compile() minus move_matmul_waits_to_ldweights (so our
    Ldweights-dedup below stays valid; generate_event_semaphores handles
    multi-wait matmuls)."""
    from concourse import inst_simplify

    nc.insert_bir_kernel_barrier_sem_inc()
    nc.generate_event_semaphores()
    nc.remove_dead_instructions_after_branch()
    nc.validate_blocks()
    nc.dce_regs()
    nc.thread_jumps()
    nc.remove_dead_blocks()
    nc.remove_dead_allocations()
    nc.verify_switch_hints()
    nc.alloc_regs()
    inst_simplify.simplify(nc)
    nc.fuse_regops()
    nc.fuse_blocks()
    nc.replace_nops_with_events()
    for engine in nc.engines:
        nc.fuse_nops(engine)
    nc.remove_dead_nops()
    nc.remove_dangling_data()
    nc.generate_event_semaphores()
    nc.insert_library_loads()
    nc.insert_act_table_loads()
    nc.insert_hostgen_rebases()
    nc.codegen_inst_isa_subclasses()
    _dedup_ldweights(nc)


def _dedup_ldweights(nc):
    """Drop PE Ldweights that reload the already-loaded stationary weights.

    codegen_inst_isa_subclasses splits every matmul into Ldweights+Matmult;
    long runs of recursion matmuls share one stationary matrix, so the
    repeated 128-row reload (~100ns each) would clog the PE stream.  Keep
    any Ldweights carrying sync waits/updates to preserve semaphores.

    fp32 matmuls get NO Ldweights (the Matmult self-loads its stationary)
    and may be scheduled anywhere in the stream — they clobber the loaded
    weights, so a Matmult whose stationary is not the tracked one resets
    the dedup state."""
    import re

    removed = 0
    for fn in nc.m.functions:
        for blk in fn.blocks:
            cur_sig = None
            out = []
            for inst in blk.instructions:
                tname = type(inst).__name__
                if tname == "InstLdweights":
                    sig = inst.concise().split("Ldweights", 1)[-1]
                    if sig == cur_sig and not inst.has_wait() \
                            and not inst.has_update():
                        removed += 1
                        continue
                    cur_sig = sig
                elif tname in ("InstMatmult", "InstMatmultMx"):
                    # stationary operand = last dt.xxx@name in in=[...]
                    ops = re.findall(r"dt\.\w+@(\w+)",
                                     inst.concise().split("in=", 1)[-1])
                    if cur_sig is None or not ops or ops[-1] not in cur_sig:
                        cur_sig = None  # self-loaded weights clobber array
                elif str(inst.engine) == "EngineType.PE" and tname not in (
                        "InstEventSemaphore", "InstNop", "InstSemWait"):
                    cur_sig = None
                out.append(inst)
            blk.instructions = out
    return removed


def prep_inputs(emissions, tags, transition_scores):
    """Host-side layout prep -> per-core input maps.

    Packed layout (per core, 256 rows):
      part = blk*32 + j, col = s*128 + b, where
      blk 0: fwd rows   0-127   value x[row=b,     t=s,     tag j]
      blk 1: fwd rows 128-255   value x[row=128+b, t=s,     tag j]
      blk 2: bwd rows   0-127   value x[row=b,     t=511-s, tag j]
      blk 3: bwd rows 128-255   value x[row=128+b, t=511-s, tag j]
    ep holds emissions (pad NEG), hp the one-hot of tags (pad 0).
    """
    e = np.asarray(emissions)
    t = np.asarray(tags)
    ep = np.full((B, S, JP), NEG, dtype=NPBF16)
    ep[:, :, :NT] = e.astype(NPBF16)
    hp = np.zeros((B, S, JP), dtype=NPBF16)
    np.put_along_axis(hp, t[..., None], np.asarray(1.0, NPBF16), axis=2)

    def pack(x):
        x = x.reshape(NCORES, 2, 128, S, JP)      # [core, half, b, s, j]
        fwd = x[:, :, :, :M, :]                   # [core, half, b, s, j]
        bwd = x[:, :, :, ::-1, :][:, :, :, :M, :]
        X = np.stack([fwd[:, 0], fwd[:, 1], bwd[:, 0], bwd[:, 1]],
                     axis=1)                      # [core, blk, b, s, j]
        X = np.ascontiguousarray(X.transpose(0, 1, 4, 3, 2))
        return X.reshape(NCORES, 128, M * FD)     # part=blk*32+j, col=s*128+b

    epk, hpk = pack(ep), pack(hp)
    tr64 = np.asarray(transition_scores, dtype=np.float64)
    etr = np.exp(tr64).astype(NPBF16)
    etrt = np.ascontiguousarray(etr.T)
    return [
        {"ep": np.ascontiguousarray(epk[c]), "hp": np.ascontiguousarray(hpk[c]),
         "etr": etr, "etrt": etrt}
        for c in range(NCORES)
    ]


def host_trans_gold(tags, transition_scores):
    """Gold transition score summed over all rows: tags-only bigram
    histogram dotted with the 24x24 table (exact, fp64)."""
    t = np.asarray(tags).astype(np.int64)
    pairs = t[:, :-1] * NT + t[:, 1:]
    counts = np.bincount(pairs.ravel(), minlength=NT * NT).astype(np.float64)
    tr64 = np.asarray(transition_scores, dtype=np.float64)
    return float((counts * tr64.ravel()).sum())


def combine(partials, trans_gold):
    return np.float32((sum(partials) - trans_gold) / B)


_PROGRAM_CACHE = {}


def kernel(emissions, tags, mask, transition_scores):
    assert np.asarray(mask).min() == 1, "kernel assumes all-ones mask"
    in_maps = prep_inputs(emissions, tags, transition_scores)
    tg = host_trans_gold(tags, transition_scores)

    if "nc" not in _PROGRAM_CACHE:
        _PROGRAM_CACHE["nc"] = build_program()
    nc = _PROGRAM_CACHE["nc"]

    res = run_bass_kernel_spmd(nc, in_maps, core_ids=list(range(NCORES)))
    partials = [float(r["out"][0, 0]) for r in res.results]
    return combine(partials, tg)


# revision 15
# speedup vs baseline: 1.7560x; 1.1938x over previous
"""CRF loss (forward-algorithm log-partition + gold-path score) on 8 Trainium2
NeuronCores — bidirectional-recursion version.

Algorithm (per batch row):
  log_den = logsumexp over tag paths; loss = mean_b(log_den - log_num).

Key structure vs the single-direction version:
  * The forward recursion over S=512 steps is split into a forward half
    (alpha from t=0) and an independent backward half (beta from t=511),
    meeting in the middle:  Z = q_{255}^T Ê r_{256}.  That halves the
    serial chain to 255 matmul->multiply rounds.
  * Both directions share ONE stationary matrix diag(Ê, Ê, Êᵀ, Êᵀ)
    (32-padded 24x24 blocks), so the PE never reloads weights:
      partitions  0- 63: forward tags for row-sets 0-127 / 128-255
      partitions 64-127: backward tags for the same row-sets
    One [128x128]@[128,128] matmul + one [128,128] DVE multiply advances
    all 256 rows of the core one step in BOTH directions.
  * Rescale: F_t = exp(e_t - c0), c0 = 6 ln2; ln Z = ln(sum z) + S*c0.
  * Gold score emission part: sum(ep * one_hot) via GpSimd multiply +
    full-tensor reduce, overlapped with the recursion.
  * Gold score transition part only needs `tags` and the 24x24 table, so
    it is computed on host from a bigram histogram (index preprocessing,
    like the one-hot encoding).

Host side reshapes/pads/one-hot-encodes inputs into the packed
[block*32+tag, step*128+col] layout (backward blocks time-reversed).
"""

import math

import numpy as np
import ml_dtypes

import concourse.bass as bass
import concourse.bacc as bacc
import concourse.tile as tile
import concourse.mybir as mybir
import concourse.bass_utils as bass_utils
from concourse.bass_utils import run_bass_kernel_spmd

BF16 = mybir.dt.bfloat16
F32 = mybir.dt.float32
AF = mybir.ActivationFunctionType
ALU = mybir.AluOpType
NPBF16 = ml_dtypes.bfloat16

B, S, NT = 2048, 512, 24
JP = 32                    # padded tag dim (multiple of 32)
NCORES = 8
RPC = B // NCORES          # rows per core (256)
M = S // 2                 # steps per direction (256)
FD = 128                   # free dim: 128 row-slots per partition-block
CS = 32                    # steps per streamed chunk
NCH = M // CS              # chunks (8)
C0 = 6.0 * math.log(2.0)   # per-step rescale of the partition chain
NEG = -30000.0             # pad value; exp -> 0


def build_program():
    nc = bacc.Bacc(trn_type="TRN2")
    ep_d = nc.dram_tensor("ep", [128, M * FD], BF16, kind="ExternalInput")
    hp_d = nc.dram_tensor("hp", [128, M * FD], BF16, kind="ExternalInput")
    etr_d = nc.dram_tensor("etr", [NT, NT], BF16, kind="ExternalInput")
    etrt_d = nc.dram_tensor("etrt", [NT, NT], BF16, kind="ExternalInput")
    out_d = nc.dram_tensor("out", [1, 1], F32, kind="ExternalOutput")

    with tile.TileContext(nc) as tc:
        with tc.tile_pool(name="const", bufs=1) as const, \
             tc.tile_pool(name="chunks", bufs=3) as chunks, \
             tc.tile_pool(name="prods", bufs=2) as prods, \
             tc.tile_pool(name="state", bufs=1) as state, \
             tc.tile_pool(name="small", bufs=4) as small, \
             tc.tile_pool(name="pmm", bufs=2, space="PSUM") as pmm, \
             tc.tile_pool(name="psg", bufs=1, space="PSUM") as psg:

            # ---- constants (memset-only parts; DMA parts deferred) ----
            wbd = const.tile([128, 128], BF16)
            nc.vector.memset(wbd[:, :], 0.0)
            sel64 = const.tile([64, 2], F32)
            nc.vector.memset(sel64[:, :], 0.0)
            nc.vector.memset(sel64[0:32, 0:1], 1.0)
            nc.vector.memset(sel64[32:64, 1:2], 1.0)
            ones2 = const.tile([2, 1], F32)
            nc.vector.memset(ones2[:, :], 1.0)
            onesn128 = const.tile([128, 1], F32)
            nc.vector.memset(onesn128[:, :], -1.0)
            negc0 = const.tile([128, 1], F32)
            nc.vector.memset(negc0[:, :], -C0)

            # ---- persistent state ----
            q = state.tile([128, FD], BF16, name="q")
            gparts = state.tile([128, NCH + 1], F32, name="gparts")
            nc.vector.memset(gparts[:, :], 0.0)

            ep_tiles = [None] * NCH
            hp_tiles = [None] * NCH
            fh_tiles = [None] * NCH

            def prep_chunk(k):
                ep_t = chunks.tile([128, CS, FD], BF16, tag="ep",
                                   name=f"ep_{k}")
                hp_t = chunks.tile([128, CS, FD], BF16, tag="hp",
                                   name=f"hp_{k}")
                lo = k * CS * FD
                nc.sync.dma_start(
                    out=ep_t[:, :, :],
                    in_=ep_d[:, lo:lo + CS * FD]
                    .rearrange("p (s c) -> p s c", c=FD))
                nc.sync.dma_start(
                    out=hp_t[:, :, :],
                    in_=hp_d[:, lo:lo + CS * FD]
                    .rearrange("p (s c) -> p s c", c=FD))
                fh = chunks.tile([128, CS, FD], BF16, tag="fh",
                                 name=f"fh_{k}")
                # F_hat = exp(ep - c0); split so early steps unblock sooner
                for e4 in range(4):
                    ssl = slice(e4 * CS // 4, (e4 + 1) * CS // 4)
                    nc.scalar.activation(fh[:, ssl, :], ep_t[:, ssl, :],
                                         AF.Exp, bias=negc0[:, :])
                ep_tiles[k] = ep_t
                hp_tiles[k] = hp_t
                fh_tiles[k] = fh

            def gold_chunk(k):
                # emission gold partial: sum(ep * hp) over the whole chunk.
                # Multiply on GpSimd; per-partition reduce on ScalarE via
                # the activation accumulator (keeps the DVE recursion clean).
                prod = prods.tile([128, CS * FD], BF16, tag="prod",
                                  name=f"prod_{k}")
                nc.gpsimd.tensor_mul(
                    prod[:, :],
                    ep_tiles[k][:, :, :].rearrange("p s c -> p (s c)"),
                    hp_tiles[k][:, :, :].rearrange("p s c -> p (s c)"))
                scr = prods.tile([128, CS * FD], BF16, tag="scr",
                                 name=f"scr_{k}")
                nc.scalar.activation(scr[:, :], prod[:, :], AF.Copy,
                                     accum_out=gparts[:, k:k + 1])

            # ---- pipeline ----
            prep_chunk(0)
            prep_chunk(1)
            # deferred constant DMAs (queue behind the chunk-0 loads that
            # gate the first recursion rounds)
            e24 = const.tile([NT, NT], BF16)
            nc.sync.dma_start(out=e24[:, :], in_=etr_d[:, :])
            et24 = const.tile([NT, NT], BF16)
            nc.sync.dma_start(out=et24[:, :], in_=etrt_d[:, :])
            for g in range(4):
                src = e24 if g < 2 else et24
                nc.sync.dma_start(
                    out=wbd[32 * g:32 * g + NT, 32 * g:32 * g + NT],
                    in_=src[:, :])
            # q init: step-0 factors for both directions
            nc.vector.tensor_copy(q[:, :], fh_tiles[0][:, 0, :])

            # Recursion as two independent column-chains (cols 0-63 /
            # 64-127) ping-ponging on PE and DVE: each half-op is smaller
            # (shorter serial latency) and the chains hide each other's
            # semaphore hops.
            HF = FD // 2
            for k in range(NCH):
                if k + 2 < NCH:
                    prep_chunk(k + 2)
                gold_chunk(k)
                s_lo = 1 if k == 0 else 0
                for sl in range(s_lo, CS):
                    pa = pmm.tile([128, HF], F32, tag="pa",
                                  name=f"pa_{k}_{sl}")
                    nc.tensor.matmul(pa[:, :], wbd[:, :], q[:, 0:HF],
                                     start=True, stop=True)
                    nc.vector.tensor_mul(q[:, 0:HF], pa[:, :],
                                         fh_tiles[k][:, sl, 0:HF])
                    pb = pmm.tile([128, HF], F32, tag="pb",
                                  name=f"pb_{k}_{sl}")
                    nc.tensor.matmul(pb[:, :], wbd[:, :], q[:, HF:FD],
                                     start=True, stop=True)
                    nc.vector.tensor_mul(q[:, HF:FD], pb[:, :],
                                         fh_tiles[k][:, sl, HF:FD])

            # ---- finalization:  Z = q_fwd^T Ê r_bwd  per row ----
            pza = pmm.tile([128, HF], F32, tag="pa", name="pza")
            nc.tensor.matmul(pza[:, :], wbd[:, :], q[:, 0:HF],
                             start=True, stop=True)
            pzb = pmm.tile([128, HF], F32, tag="pb", name="pzb")
            nc.tensor.matmul(pzb[:, :], wbd[:, :], q[:, HF:FD],
                             start=True, stop=True)
            qsh = small.tile([64, FD], BF16, tag="qsh", name="qsh")
            nc.sync.dma_start(out=qsh[:, :], in_=q[64:128, :])
            z = small.tile([64, FD], F32, tag="z", name="z")
            nc.vector.tensor_mul(z[:, 0:HF], pza[0:64, :], qsh[:, 0:HF])
            nc.vector.tensor_mul(z[:, HF:FD], pzb[0:64, :], qsh[:, HF:FD])
            zz = psg.tile([2, FD], F32, tag="fin", name="zz")
            nc.tensor.matmul(zz[:, :], sel64[:, :], z[:, :],
                             start=True, stop=True)
            lnz = small.tile([2, FD], F32, tag="lnz", name="lnz")
            nc.scalar.activation(lnz[:, :], zz[:, :], AF.Ln)
            lnr = small.tile([2, 1], F32, tag="lnr", name="lnr")
            nc.vector.reduce_sum(lnr[:, :], lnz[:, :],
                                 axis=mybir.AxisListType.X)
            zt = psg.tile([1, 1], F32, tag="finzt", name="zt")
            nc.tensor.matmul(zt[:, :], ones2[:, :], lnr[:, :],
                             start=True, stop=True)

            # zero-write depending on the final q: pins the gold
            # finalization to the END of the in-order DVE/PE queues (the
            # scheduler's GpSimd timing model is optimistic; without this
            # it hoists g128/fing mid-stream and head-of-line blocks the
            # recursion behind slow gold accumulates).
            nc.vector.tensor_scalar_mul(gparts[:, NCH:NCH + 1], q[:, 0:1],
                                        0.0)
            g128 = small.tile([128, 1], F32, tag="g128", name="g128")
            nc.vector.reduce_sum(g128[:, :], gparts[:, :],
                                 axis=mybir.AxisListType.X)
            fing = psg.tile([1, 1], F32, tag="fin2", name="fing")
            nc.tensor.matmul(fing[:, :], onesn128[:, :], g128[:, :],
                             start=True, stop=True)

            sl_t = small.tile([1, 1], F32, tag="outv", name="sl_t")
            nc.scalar.copy(sl_t[:, :], zt[:, :])
            sg_t = small.tile([1, 1], F32, tag="outv", name="sg_t")
            nc.scalar.copy(sg_t[:, :], fing[:, :])
            outv = small.tile([1, 1], F32, tag="outv", name="outv")
            # + per-core constant: RPC rows * S steps * c0 (sg holds -gold)
            nc.vector.scalar_tensor_tensor(
                outv[:, :], sl_t[:, :], float(RPC) * float(S) * C0,
                sg_t[:, :], ALU.add, ALU.add)
            nc.sync.dma_start(out=out_d[:, :], in_=outv[:, :])
    _bacc_compile_no_ldw_split(nc)
    return nc


def _bacc_compile_no_ldw_split(nc):
    """Bacc.compile() minus move_matmul_waits_to_ldweights (so our
    Ldweights-dedup below stays valid; generate_event_semaphores handles
    multi-wait matmuls)."""
    from concourse import inst_simplify

    nc.insert_bir_kernel_barrier_sem_inc()
    nc.generate_event_semaphores()
    nc.remove_dead_instructions_after_branch()
    nc.validate_blocks()
    nc.dce_regs()
    nc.thread_jumps()
    nc.remove_dead_blocks()
    nc.remove_dead_allocations()
    nc.verify_switch_hints()
    nc.alloc_regs()
    inst_simplify.simplify(nc)
    nc.fuse_regops()
    nc.fuse_blocks()
    nc.replace_nops_with_events()
    for engine in nc.engines:
        nc.fuse_nops(engine)
    nc.remove_dead_nops()
    nc.remove_dangling_data()
    nc.generate_event_semaphores()
    nc.insert_library_loads()
    nc.insert_act_table_loads()
    nc.insert_hostgen_rebases()
    nc.codegen_inst_isa_subclasses()
    _dedup_ldweights(nc)


def _dedup_ldweights(nc):
    """Drop PE Ldweights that reload the already-loaded stationary weights.

    codegen_inst_isa_subclasses splits every matmul into Ldweights+Matmult;
    long runs of recursion matmuls share one stationary matrix, so the
    repeated 128-row reload (~100ns each) would clog the PE stream.  Keep
    any Ldweights carrying sync waits/updates to preserve semaphores.

    fp32 matmuls get NO Ldweights (the Matmult self-loads its stationary)
    and may be scheduled anywhere in the stream — they clobber the loaded
    weights, so a Matmult whose stationary is not the tracked one resets
    the dedup state."""
    import re

    removed = 0
    for fn in nc.m.functions:
        for blk in fn.blocks:
            cur_sig = None
            out = []
            for inst in blk.instructions:
                tname = type(inst).__name__
                if tname == "InstLdweights":
                    sig = inst.concise().split("Ldweights", 1)[-1]
                    if sig == cur_sig and not inst.has_wait() \
                            and not inst.has_update():
                        removed += 1
                        continue
                    cur_sig = sig
                elif tname in ("InstMatmult", "InstMatmultMx"):
                    # stationary operand = last dt.xxx@name in in=[...]
                    ops = re.findall(r"dt\.\w+@(\w+)",
                                     inst.concise().split("in=", 1)[-1])
                    if cur_sig is None or not ops or ops[-1] not in cur_sig:
                        cur_sig = None  # self-loaded weights clobber array
                elif str(inst.engine) == "EngineType.PE" and tname not in (
                        "InstEventSemaphore", "InstNop", "InstSemWait"):
                    cur_sig = None
                out.append(inst)
            blk.instructions = out
    return removed


def prep_inputs(emissions, tags, transition_scores):
    """Host-side layout prep -> per-core input maps.

    Packed layout (per core, 256 rows):
      part = blk*32 + j, col = s*128 + b, where
      blk 0: fwd rows   0-127   value x[row=b,     t=s,     tag j]
      blk 1: fwd rows 128-255   value x[row=128+b, t=s,     tag j]
      blk 2: bwd rows   0-127   value x[row=b,     t=511-s, tag j]
      blk 3: bwd rows 128-255   value x[row=128+b, t=511-s, tag j]
    ep holds emissions (pad NEG), hp the one-hot of tags (pad 0).
    """
    e = np.asarray(emissions)
    t = np.asarray(tags)
    ep = np.full((B, S, JP), NEG, dtype=NPBF16)
    ep[:, :, :NT] = e.astype(NPBF16)
    hp = np.zeros((B, S, JP), dtype=NPBF16)
    np.put_along_axis(hp, t[..., None], np.asarray(1.0, NPBF16), axis=2)

    def pack(x):
        x = x.reshape(NCORES, 2, 128, S, JP)      # [core, half, b, s, j]
        fwd = x[:, :, :, :M, :]                   # [core, half, b, s, j]
        bwd = x[:, :, :, ::-1, :][:, :, :, :M, :]
        X = np.stack([fwd[:, 0], fwd[:, 1], bwd[:, 0], bwd[:, 1]],
                     axis=1)                      # [core, blk, b, s, j]
        X = np.ascontiguousarray(X.transpose(0, 1, 4, 3, 2))
        return X.reshape(NCORES, 128, M * FD)     # part=blk*32+j, col=s*128+b

    epk, hpk = pack(ep), pack(hp)
    tr64 = np.asarray(transition_scores, dtype=np.float64)
    etr = np.exp(tr64).astype(NPBF16)
    etrt = np.ascontiguousarray(etr.T)
    return [
        {"ep": np.ascontiguousarray(epk[c]), "hp": np.ascontiguousarray(hpk[c]),
         "etr": etr, "etrt": etrt}
        for c in range(NCORES)
    ]


def host_trans_gold(tags, transition_scores):
    """Gold transition score summed over all rows: tags-only bigram
    histogram dotted with the 24x24 table (exact, fp64)."""
    t = np.asarray(tags).astype(np.int64)
    pairs = t[:, :-1] * NT + t[:, 1:]
    counts = np.bincount(pairs.ravel(), minlength=NT * NT).astype(np.float64)
    tr64 = np.asarray(transition_scores, dtype=np.float64)
    return float((counts * tr64.ravel()).sum())


def combine(partials, trans_gold):
    return np.float32((sum(partials) - trans_gold) / B)


_PROGRAM_CACHE = {}


def kernel(emissions, tags, mask, transition_scores):
    assert np.asarray(mask).min() == 1, "kernel assumes all-ones mask"
    in_maps = prep_inputs(emissions, tags, transition_scores)
    tg = host_trans_gold(tags, transition_scores)

    if "nc" not in _PROGRAM_CACHE:
        _PROGRAM_CACHE["nc"] = build_program()
    nc = _PROGRAM_CACHE["nc"]

    res = run_bass_kernel_spmd(nc, in_maps, core_ids=list(range(NCORES)))
    partials = [float(r["out"][0, 0]) for r in res.results]
    return combine(partials, tg)


# revision 19
# speedup vs baseline: 1.8609x; 1.0597x over previous
"""CRF loss (forward-algorithm log-partition + gold-path score) on 8 Trainium2
NeuronCores — bidirectional-recursion version.

Algorithm (per batch row):
  log_den = logsumexp over tag paths; loss = mean_b(log_den - log_num).

Key structure vs the single-direction version:
  * The forward recursion over S=512 steps is split into a forward half
    (alpha from t=0) and an independent backward half (beta from t=511),
    meeting in the middle:  Z = q_{255}^T Ê r_{256}.  That halves the
    serial chain to 255 matmul->multiply rounds.
  * Both directions share ONE stationary matrix diag(Ê, Ê, Êᵀ, Êᵀ)
    (32-padded 24x24 blocks), so the PE never reloads weights:
      partitions  0- 63: forward tags for row-sets 0-127 / 128-255
      partitions 64-127: backward tags for the same row-sets
    One [128x128]@[128,128] matmul + one [128,128] DVE multiply advances
    all 256 rows of the core one step in BOTH directions.
  * Rescale: F_t = exp(e_t - c0), c0 = 6 ln2; ln Z = ln(sum z) + S*c0.
  * Gold score emission part: sum(ep * one_hot) via GpSimd multiply +
    full-tensor reduce, overlapped with the recursion.
  * Gold score transition part only needs `tags` and the 24x24 table, so
    it is computed on host from a bigram histogram (index preprocessing,
    like the one-hot encoding).

Host side reshapes/pads/one-hot-encodes inputs into the packed
[block*32+tag, step*128+col] layout (backward blocks time-reversed).
"""

import math

import numpy as np
import ml_dtypes

import concourse.bass as bass
import concourse.bacc as bacc
import concourse.tile as tile
import concourse.mybir as mybir
import concourse.bass_utils as bass_utils
from concourse.bass_utils import run_bass_kernel_spmd

BF16 = mybir.dt.bfloat16
F32 = mybir.dt.float32
AF = mybir.ActivationFunctionType
ALU = mybir.AluOpType
NPBF16 = ml_dtypes.bfloat16

B, S, NT = 2048, 512, 24
JP = 32                    # padded tag dim (multiple of 32)
NCORES = 8
RPC = B // NCORES          # rows per core (256)
M = S // 2                 # steps per direction (256)
FD = 128                   # free dim: 128 row-slots per partition-block
CS = 32                    # steps per streamed chunk
NCH = M // CS              # chunks (8)
C0 = 6.0 * math.log(2.0)   # per-step rescale of the partition chain
NEG = -30000.0             # pad value; exp -> 0


def build_program():
    nc = bacc.Bacc(trn_type="TRN2")
    ep_d = nc.dram_tensor("ep", [128, M * FD], BF16, kind="ExternalInput")
    hp_d = nc.dram_tensor("hp", [128, M * FD], BF16, kind="ExternalInput")
    wbd_d = nc.dram_tensor("wbd", [128, 128], BF16, kind="ExternalInput")
    out_d = nc.dram_tensor("out", [1, 1], F32, kind="ExternalOutput")

    with tile.TileContext(nc) as tc:
        with tc.tile_pool(name="const", bufs=1) as const, \
             tc.tile_pool(name="chunks", bufs=3) as chunks, \
             tc.tile_pool(name="prods", bufs=2) as prods, \
             tc.tile_pool(name="state", bufs=1) as state, \
             tc.tile_pool(name="small", bufs=4) as small, \
             tc.tile_pool(name="pmm", bufs=2, space="PSUM") as pmm, \
             tc.tile_pool(name="psg", bufs=1, space="PSUM") as psg:

            # ---- constants ----
            # stationary diag(Ê,Ê,Êᵀ,Êᵀ) built on host; first DMA in queue
            wbd = const.tile([128, 128], BF16)
            nc.sync.dma_start(out=wbd[:, :], in_=wbd_d[:, :])
            # dummy activation so the Exp act-table load schedules at the
            # very start (before any DMA-dependent activation)
            dumm = const.tile([1, 1], F32)
            nc.vector.memset(dumm[:, :], 0.0)
            dumo = const.tile([1, 1], F32)
            nc.scalar.activation(dumo[:, :], dumm[:, :], AF.Exp)
            sel64 = const.tile([64, 2], F32)
            nc.vector.memset(sel64[:, :], 0.0)
            nc.vector.memset(sel64[0:32, 0:1], 1.0)
            nc.vector.memset(sel64[32:64, 1:2], 1.0)
            ones2 = const.tile([2, 1], F32)
            nc.vector.memset(ones2[:, :], 1.0)
            onesn128 = const.tile([128, 1], F32)
            nc.vector.memset(onesn128[:, :], -1.0)
            negc0 = const.tile([128, 1], F32)
            nc.vector.memset(negc0[:, :], -C0)

            # ---- persistent state ----
            q = state.tile([128, FD], BF16, name="q")
            gparts = state.tile([128, NCH + 1], F32, name="gparts")
            nc.vector.memset(gparts[:, :], 0.0)

            ep_tiles = [None] * NCH
            hp_tiles = [None] * NCH
            fh_tiles = [None] * NCH

            def prep_chunk(k):
                ep_t = chunks.tile([128, CS, FD], BF16, tag="ep",
                                   name=f"ep_{k}")
                hp_t = chunks.tile([128, CS, FD], BF16, tag="hp",
                                   name=f"hp_{k}")
                lo = k * CS * FD
                nc.sync.dma_start(
                    out=ep_t[:, :, :],
                    in_=ep_d[:, lo:lo + CS * FD]
                    .rearrange("p (s c) -> p s c", c=FD))
                nc.sync.dma_start(
                    out=hp_t[:, :, :],
                    in_=hp_d[:, lo:lo + CS * FD]
                    .rearrange("p (s c) -> p s c", c=FD))
                fh = chunks.tile([128, CS, FD], BF16, tag="fh",
                                 name=f"fh_{k}")
                # F_hat = exp(ep - c0); split so early steps unblock sooner
                for e4 in range(4):
                    ssl = slice(e4 * CS // 4, (e4 + 1) * CS // 4)
                    nc.scalar.activation(fh[:, ssl, :], ep_t[:, ssl, :],
                                         AF.Exp, bias=negc0[:, :])
                ep_tiles[k] = ep_t
                hp_tiles[k] = hp_t
                fh_tiles[k] = fh

            def gold_chunk(k):
                # emission gold partial: sum(ep * hp) over the whole chunk.
                # Multiply on GpSimd; per-partition reduce on ScalarE via
                # the activation accumulator (keeps the DVE recursion clean).
                prod = prods.tile([128, CS * FD], BF16, tag="prod",
                                  name=f"prod_{k}")
                nc.gpsimd.tensor_mul(
                    prod[:, :],
                    ep_tiles[k][:, :, :].rearrange("p s c -> p (s c)"),
                    hp_tiles[k][:, :, :].rearrange("p s c -> p (s c)"))
                scr = prods.tile([128, CS * FD], BF16, tag="scr",
                                 name=f"scr_{k}")
                nc.scalar.activation(scr[:, :], prod[:, :], AF.Copy,
                                     accum_out=gparts[:, k:k + 1])

            # ---- pipeline ----
            prep_chunk(0)
            prep_chunk(1)
            # q init: step-0 factors for both directions
            nc.vector.tensor_copy(q[:, :], fh_tiles[0][:, 0, :])

            # Recursion as two independent column-chains (cols 0-63 /
            # 64-127) ping-ponging on PE and DVE: each half-op is smaller
            # (shorter serial latency) and the chains hide each other's
            # semaphore hops.
            HF = FD // 2
            for k in range(NCH):
                if k + 2 < NCH:
                    prep_chunk(k + 2)
                gold_chunk(k)
                s_lo = 1 if k == 0 else 0
                for sl in range(s_lo, CS):
                    pa = pmm.tile([128, HF], F32, tag="pa",
                                  name=f"pa_{k}_{sl}")
                    nc.tensor.matmul(pa[:, :], wbd[:, :], q[:, 0:HF],
                                     start=True, stop=True)
                    nc.vector.tensor_mul(q[:, 0:HF], pa[:, :],
                                         fh_tiles[k][:, sl, 0:HF])
                    pb = pmm.tile([128, HF], F32, tag="pb",
                                  name=f"pb_{k}_{sl}")
                    nc.tensor.matmul(pb[:, :], wbd[:, :], q[:, HF:FD],
                                     start=True, stop=True)
                    nc.vector.tensor_mul(q[:, HF:FD], pb[:, :],
                                         fh_tiles[k][:, sl, HF:FD])

            # ---- finalization:  Z = q_fwd^T Ê r_bwd  per row ----
            pza = pmm.tile([128, HF], F32, tag="pa", name="pza")
            nc.tensor.matmul(pza[:, :], wbd[:, :], q[:, 0:HF],
                             start=True, stop=True)
            pzb = pmm.tile([128, HF], F32, tag="pb", name="pzb")
            nc.tensor.matmul(pzb[:, :], wbd[:, :], q[:, HF:FD],
                             start=True, stop=True)
            qsh = small.tile([64, FD], BF16, tag="qsh", name="qsh")
            nc.sync.dma_start(out=qsh[:, :], in_=q[64:128, :])
            z = small.tile([64, FD], F32, tag="z", name="z")
            nc.vector.tensor_mul(z[:, 0:HF], pza[0:64, :], qsh[:, 0:HF])
            nc.vector.tensor_mul(z[:, HF:FD], pzb[0:64, :], qsh[:, HF:FD])
            zz = psg.tile([2, FD], F32, tag="fin", name="zz")
            nc.tensor.matmul(zz[:, :], sel64[:, :], z[:, :],
                             start=True, stop=True)
            lnz = small.tile([2, FD], F32, tag="lnz", name="lnz")
            nc.scalar.activation(lnz[:, :], zz[:, :], AF.Ln)
            lnr = small.tile([2, 1], F32, tag="lnr", name="lnr")
            nc.vector.reduce_sum(lnr[:, :], lnz[:, :],
                                 axis=mybir.AxisListType.X)
            zt = psg.tile([1, 1], F32, tag="finzt", name="zt")
            nc.tensor.matmul(zt[:, :], ones2[:, :], lnr[:, :],
                             start=True, stop=True)

            # zero-write depending on the final q: pins the gold
            # finalization to the END of the in-order DVE/PE queues (the
            # scheduler's GpSimd timing model is optimistic; without this
            # it hoists g128/fing mid-stream and head-of-line blocks the
            # recursion behind slow gold accumulates).
            nc.vector.tensor_scalar_mul(gparts[:, NCH:NCH + 1], q[:, 0:1],
                                        0.0)
            g128 = small.tile([128, 1], F32, tag="g128", name="g128")
            nc.vector.reduce_sum(g128[:, :], gparts[:, :],
                                 axis=mybir.AxisListType.X)
            fing = psg.tile([1, 1], F32, tag="fin2", name="fing")
            nc.tensor.matmul(fing[:, :], onesn128[:, :], g128[:, :],
                             start=True, stop=True)

            sl_t = small.tile([1, 1], F32, tag="outv", name="sl_t")
            nc.scalar.copy(sl_t[:, :], zt[:, :])
            sg_t = small.tile([1, 1], F32, tag="outv", name="sg_t")
            nc.scalar.copy(sg_t[:, :], fing[:, :])
            outv = small.tile([1, 1], F32, tag="outv", name="outv")
            # + per-core constant: RPC rows * S steps * c0 (sg holds -gold)
            nc.vector.scalar_tensor_tensor(
                outv[:, :], sl_t[:, :], float(RPC) * float(S) * C0,
                sg_t[:, :], ALU.add, ALU.add)
            nc.sync.dma_start(out=out_d[:, :], in_=outv[:, :])
    _bacc_compile_no_ldw_split(nc)
    return nc


def _bacc_compile_no_ldw_split(nc):
    """Bacc.compile() minus move_matmul_waits_to_ldweights (so our
    Ldweights-dedup below stays valid; generate_event_semaphores handles
    multi-wait matmuls)."""
    from concourse import inst_simplify

    nc.insert_bir_kernel_barrier_sem_inc()
    nc.generate_event_semaphores()
    nc.remove_dead_instructions_after_branch()
    nc.validate_blocks()
    nc.dce_regs()
    nc.thread_jumps()
    nc.remove_dead_blocks()
    nc.remove_dead_allocations()
    nc.verify_switch_hints()
    nc.alloc_regs()
    inst_simplify.simplify(nc)
    nc.fuse_regops()
    nc.fuse_blocks()
    nc.replace_nops_with_events()
    for engine in nc.engines:
        nc.fuse_nops(engine)
    nc.remove_dead_nops()
    nc.remove_dangling_data()
    nc.generate_event_semaphores()
    nc.insert_library_loads()
    nc.insert_act_table_loads()
    nc.insert_hostgen_rebases()
    nc.codegen_inst_isa_subclasses()
    _dedup_ldweights(nc)


def _dedup_ldweights(nc):
    """Drop PE Ldweights that reload the already-loaded stationary weights.

    codegen_inst_isa_subclasses splits every matmul into Ldweights+Matmult;
    long runs of recursion matmuls share one stationary matrix, so the
    repeated 128-row reload (~100ns each) would clog the PE stream.  Keep
    any Ldweights carrying sync waits/updates to preserve semaphores.

    fp32 matmuls get NO Ldweights (the Matmult self-loads its stationary)
    and may be scheduled anywhere in the stream — they clobber the loaded
    weights, so a Matmult whose stationary is not the tracked one resets
    the dedup state."""
    import re

    removed = 0
    for fn in nc.m.functions:
        for blk in fn.blocks:
            cur_sig = None
            out = []
            for inst in blk.instructions:
                tname = type(inst).__name__
                if tname == "InstLdweights":
                    sig = inst.concise().split("Ldweights", 1)[-1]
                    if sig == cur_sig and not inst.has_wait() \
                            and not inst.has_update():
                        removed += 1
                        continue
                    cur_sig = sig
                elif tname in ("InstMatmult", "InstMatmultMx"):
                    # stationary operand = last dt.xxx@name in in=[...]
                    ops = re.findall(r"dt\.\w+@(\w+)",
                                     inst.concise().split("in=", 1)[-1])
                    if cur_sig is None or not ops or ops[-1] not in cur_sig:
                        cur_sig = None  # self-loaded weights clobber array
                elif str(inst.engine) == "EngineType.PE" and tname not in (
                        "InstEventSemaphore", "InstNop", "InstSemWait"):
                    cur_sig = None
                out.append(inst)
            blk.instructions = out
    return removed


def prep_inputs(emissions, tags, transition_scores):
    """Host-side layout prep -> per-core input maps.

    Packed layout (per core, 256 rows):
      part = blk*32 + j, col = s*128 + b, where
      blk 0: fwd rows   0-127   value x[row=b,     t=s,     tag j]
      blk 1: fwd rows 128-255   value x[row=128+b, t=s,     tag j]
      blk 2: bwd rows   0-127   value x[row=b,     t=511-s, tag j]
      blk 3: bwd rows 128-255   value x[row=128+b, t=511-s, tag j]
    ep holds emissions (pad NEG), hp the one-hot of tags (pad 0).
    """
    e = np.asarray(emissions)
    t = np.asarray(tags)
    ep = np.full((B, S, JP), NEG, dtype=NPBF16)
    ep[:, :, :NT] = e.astype(NPBF16)
    hp = np.zeros((B, S, JP), dtype=NPBF16)
    np.put_along_axis(hp, t[..., None], np.asarray(1.0, NPBF16), axis=2)

    def pack(x):
        x = x.reshape(NCORES, 2, 128, S, JP)      # [core, half, b, s, j]
        fwd = x[:, :, :, :M, :]                   # [core, half, b, s, j]
        bwd = x[:, :, :, ::-1, :][:, :, :, :M, :]
        X = np.stack([fwd[:, 0], fwd[:, 1], bwd[:, 0], bwd[:, 1]],
                     axis=1)                      # [core, blk, b, s, j]
        X = np.ascontiguousarray(X.transpose(0, 1, 4, 3, 2))
        return X.reshape(NCORES, 128, M * FD)     # part=blk*32+j, col=s*128+b

    epk, hpk = pack(ep), pack(hp)
    tr64 = np.asarray(transition_scores, dtype=np.float64)
    etr = np.exp(tr64).astype(NPBF16)
    wbd = np.zeros((128, 128), dtype=NPBF16)
    for g in range(4):
        blk = etr if g < 2 else etr.T
        wbd[32 * g:32 * g + NT, 32 * g:32 * g + NT] = blk
    return [
        {"ep": np.ascontiguousarray(epk[c]), "hp": np.ascontiguousarray(hpk[c]),
         "wbd": wbd}
        for c in range(NCORES)
    ]


def host_trans_gold(tags, transition_scores):
    """Gold transition score summed over all rows: tags-only bigram
    histogram dotted with the 24x24 table (exact, fp64)."""
    t = np.asarray(tags).astype(np.int64)
    pairs = t[:, :-1] * NT + t[:, 1:]
    counts = np.bincount(pairs.ravel(), minlength=NT * NT).astype(np.float64)
    tr64 = np.asarray(transition_scores, dtype=np.float64)
    return float((counts * tr64.ravel()).sum())


def combine(partials, trans_gold):
    return np.float32((sum(partials) - trans_gold) / B)


_PROGRAM_CACHE = {}


def kernel(emissions, tags, mask, transition_scores):
    assert np.asarray(mask).min() == 1, "kernel assumes all-ones mask"
    in_maps = prep_inputs(emissions, tags, transition_scores)
    tg = host_trans_gold(tags, transition_scores)

    if "nc" not in _PROGRAM_CACHE:
        _PROGRAM_CACHE["nc"] = build_program()
    nc = _PROGRAM_CACHE["nc"]

    res = run_bass_kernel_spmd(nc, in_maps, core_ids=list(range(NCORES)))
    partials = [float(r["out"][0, 0]) for r in res.results]
    return combine(partials, tg)
